# revision 14
# baseline (speedup 1.0000x reference)
"""GCNN (batched SpMM + GEMM + bias + ReLU) Trainium2 kernel.

Strategy: dense block-streamed SpMM with a mixed uint8/bf16 A^T stream
(no gather, no per-edge DMA descriptors).

Per-core work (one graph per NeuronCore, 8 graphs / 8 cores):
  phase 0: y = x @ W'           (bf16 PE matmuls, y tiles stay in SBUF;
           W' = W * s/255 folds the A-dequant scale so the uint8->bf16
           A^T conversion on-chip is a pure cast)
  phase 1: out^T = A^T-blocks streamed dense:
      out^T[ch, dest] = sum_k y_k^T @ A^T[k-block, dest-block]
    - A^T built dense on host in a pass-major layout [128 src-lane,
      pass | k | dest-col]; k-blocks < KU8 are stored uint8
      (round(v*255/s), s = per-graph max) and dequant-cast to bf16
      on-chip (ACT + DVE, ~2:1), the rest are stored bf16 and used
      directly.  The split ratio balances DMA bytes (~133 MB) against
      the measured cast rates (ACT ~10us, DVE ~22us per 1.57M-elem
      slab) and the PE stream (~390us) so all engines run ~350-400us.
    - the last src block (16 real rows of 128) is streamed as a
      [16, span] bf16 slab - its 112 zero partitions are never moved
    - y_k (bf16, SBUF-resident) is the matmul *stationary* operand;
      bf16 A^T slabs are the *moving* operand
    - PSUM accumulates out^T per dest pass over all 79 k-blocks;
      7 dest passes of 12 groups (3 PSUM banks each) ping-pong across
      6 PSUM banks so evictions overlap the next pass's matmuls
    - eviction: single ACT op relu(psum + bias) (bias is per-partition
      in the out^T orientation) to bf16, DMA out; host transposes and
      upcasts

Quantization error: uint8 absolute quantization of v in [0,1) gives
~0.4% output rel err (vs 2e-2 gate); fp8 e4m3 *relative* quantization
was measured at 2.4e-2 and rejected.  Integers 0..255 are exact in
bf16, so the cast adds no further error.

SPMD: one NEFF for all 8 cores; per-core tensors differ only in data.
"""

import sys

if "/opt/trn_rl_repo" not in sys.path:
    sys.path.insert(0, "/opt/trn_rl_repo")

import numpy as np
import ml_dtypes

import concourse.bacc as bacc
import concourse.mybir as mybir
from concourse import tile
from concourse.bass_utils import run_bass_kernel_spmd

BF16 = ml_dtypes.bfloat16

C = 128            # channels (C_IN == C_OUT == 128)
N = 10000          # nodes per graph
NB = (N + 127) // 128          # 79 src blocks
KLAST = NB - 1                 # last src block: only 16 real rows
KLAST_ROWS = N - KLAST * 128   # 16
KU8 = 72                       # k-blocks [0, KU8) stored uint8, rest bf16
GROUPS_PER_PASS = 12           # 3 PSUM banks per pass, 6 banks ping-pong
BANK_COLS = 512                # fp32 columns per PSUM bank
KCHUNK = 8                     # k-blocks per DMA slab


def _passes():
    out = []
    g = 0
    while g * 128 < N:
        c0 = g * 128
        c1 = min((g + GROUPS_PER_PASS) * 128, N)
        out.append((c0, c1 - c0))          # (first dest col, real span)
        g += GROUPS_PER_PASS
    return out


PASSES = _passes()                          # [(col0, span), ...]
PASS_OFF_U8 = np.concatenate(
    [[0], np.cumsum([KU8 * s for _, s in PASSES])]).astype(np.int64)
ATQ_COLS = int(PASS_OFF_U8[-1])             # 56 * 10000
PASS_OFF_B16 = np.concatenate(
    [[0], np.cumsum([(KLAST - KU8) * s for _, s in PASSES])]).astype(np.int64)
ATB_COLS = int(PASS_OFF_B16[-1])            # 22 * 10000
# the short k=78 slab lives in a separate [16, N] tensor, pass-major too
PASS_OFF2 = np.concatenate(
    [[0], np.cumsum([s for _, s in PASSES])]).astype(np.int64)


# ---------------------------------------------------------------- host prep

def prep_in_maps(x, edge_rows, edge_cols, edge_vals, W, b):
    """Build per-core input maps: xT (bf16), W' (bf16, scale-folded),
    bT (f32 bias in out^T orientation), ATQ (uint8) / ATB (bf16) /
    AT2 (bf16) dense A^T pieces, pass-major."""
    x = np.asarray(x)
    W32 = np.asarray(W, dtype=np.float32)
    bT = np.ascontiguousarray(
        np.asarray(b, dtype=np.float32)[:, None] * np.ones((1, 1), np.float32))

    spans = np.array([s for _, s in PASSES], dtype=np.int64)

    in_maps = []
    for g in range(x.shape[0]):
        rows = np.asarray(edge_rows[g], dtype=np.int64)
        cols = np.asarray(edge_cols[g], dtype=np.int64)
        vals = np.asarray(edge_vals[g], dtype=np.float32)
        k = cols // 128
        c_loc = cols % 128
        p = rows // (GROUPS_PER_PASS * 128)
        r_rel = rows - p * GROUPS_PER_PASS * 128

        mq = k < KU8
        atq = np.zeros(C * ATQ_COLS, dtype=np.float32)
        idx = (c_loc[mq] * ATQ_COLS
               + PASS_OFF_U8[p[mq]] + k[mq] * spans[p[mq]] + r_rel[mq])
        np.add.at(atq, idx, vals[mq])

        mb = (k >= KU8) & (k < KLAST)
        atb = np.zeros(C * ATB_COLS, dtype=np.float32)
        idx = (c_loc[mb] * ATB_COLS
               + PASS_OFF_B16[p[mb]] + (k[mb] - KU8) * spans[p[mb]]
               + r_rel[mb])
        np.add.at(atb, idx, vals[mb])

        m2 = k == KLAST
        at2 = np.zeros(KLAST_ROWS * N, dtype=np.float32)
        idx2 = c_loc[m2] * N + PASS_OFF2[p[m2]] + r_rel[m2]
        np.add.at(at2, idx2, vals[m2])

        s = float(max(atq.max(), 1e-9))
        q = np.clip(np.rint(atq * (255.0 / s)), 0, 255).astype(np.uint8)
        sc = np.float32(s / 255.0)

        in_maps.append({
            "xT": np.ascontiguousarray(x[g].T.astype(BF16)),
            "W": (W32 * sc).astype(BF16),
            "bT": bT,
            "ATQ": q.reshape(C, ATQ_COLS),
            # bf16 pieces carry real values; pre-divide by the folded
            # scale so one W' serves both streams
            "ATB": (atb.reshape(C, ATB_COLS) / sc).astype(BF16),
            "AT2": (at2.reshape(KLAST_ROWS, N) / sc).astype(BF16),
        })
    return in_maps


# ---------------------------------------------------------------- device IR

def build_nc():
    f32 = mybir.dt.float32
    bf16 = mybir.dt.bfloat16
    u8 = mybir.dt.uint8

    nc = bacc.Bacc("TRN2")
    xT_d = nc.dram_tensor("xT", [C, N], bf16, kind="ExternalInput")
    W_d = nc.dram_tensor("W", [C, C], bf16, kind="ExternalInput")
    bT_d = nc.dram_tensor("bT", [C, 1], f32, kind="ExternalInput")
    ATQ_d = nc.dram_tensor("ATQ", [C, ATQ_COLS], u8, kind="ExternalInput")
    ATB_d = nc.dram_tensor("ATB", [C, ATB_COLS], bf16, kind="ExternalInput")
    AT2_d = nc.dram_tensor("AT2", [KLAST_ROWS, N], bf16, kind="ExternalInput")
    outT_d = nc.dram_tensor("outT", [C, N], bf16, kind="ExternalOutput")

    max_slab = KCHUNK * GROUPS_PER_PASS * 128   # elems per partition

    with tile.TileContext(nc) as tc:
        with (
            tc.tile_pool(name="const", bufs=1) as constp,
            tc.tile_pool(name="ypool", bufs=NB) as ypool,
            tc.tile_pool(name="p0ps", bufs=2, space="PSUM") as p0ps,
            tc.tile_pool(name="xp", bufs=3) as xp,
            tc.tile_pool(name="atq", bufs=4) as atqp,
            tc.tile_pool(name="atp", bufs=4) as atp,
            tc.tile_pool(name="at2p", bufs=2) as at2p,
            tc.tile_pool(name="acc", bufs=6, space="PSUM") as accp,
            tc.tile_pool(name="ev", bufs=3) as evp,
        ):
            # ---- constants
            w_t = constp.tile([C, C], bf16, tag="w")
            nc.sync.dma_start(out=w_t[:], in_=W_d[:])
            bias_t = constp.tile([C, 1], f32, tag="bias")
            nc.sync.dma_start(out=bias_t[:], in_=bT_d[:])
            # ---- phase 0: y = x @ W', tiles kept resident in SBUF (bf16)
            y_tiles = []
            for t in range(NB):
                rows = min(128, N - t * 128)
                x_t = xp.tile([C, 128], bf16, tag="xt")
                nc.scalar.dma_start(out=x_t[:, :rows],
                                    in_=xT_d[:, t * 128:t * 128 + rows])
                yps = p0ps.tile([128, C], f32, tag="yps")
                nc.tensor.matmul(yps[:rows, :],
                                 x_t[:, :rows],
                                 w_t[:], start=True, stop=True)
                ysb = ypool.tile([128, C], bf16, tag="y", name=f"y_{t}")
                nc.vector.tensor_copy(ysb[:rows, :], yps[:rows, :])
                y_tiles.append(ysb)

            # ---- phase 1: stream A^T slabs (u8 -> cast, or bf16 direct),
            #      accumulate out^T in PSUM
            dma_engines = [nc.sync, nc.scalar]
            slab_i = 0
            u8_i = 0
            for pi, (col0, span) in enumerate(PASSES):
                nbank = (span + BANK_COLS - 1) // BANK_COLS
                ps = []
                for bi in range(nbank):
                    pt = accp.tile([128, BANK_COLS], f32, tag="acc",
                                   name=f"acc_{col0}_{bi}")
                    ps.append(pt)
                for k0 in range(0, KLAST, KCHUNK):
                    if k0 < KU8:
                        kn = min(KCHUNK, KU8 - k0)
                        atq_t = atqp.tile([128, max_slab], u8, tag="atq")
                        lo = int(PASS_OFF_U8[pi]) + k0 * span
                        dma_engines[slab_i % 2].dma_start(
                            out=atq_t[:, :kn * span],
                            in_=ATQ_d[:, lo:lo + kn * span])
                        at_t = atp.tile([128, max_slab], bf16, tag="at")
                        # dequant-cast: alternate ACT / DVE (similar rates
                        # when GPSIMD stays off the shared SBUF port)
                        if u8_i % 2 == 1:
                            nc.vector.tensor_copy(at_t[:, :kn * span],
                                                  atq_t[:, :kn * span])
                        else:
                            nc.scalar.activation(
                                out=at_t[:, :kn * span],
                                in_=atq_t[:, :kn * span],
                                func=mybir.ActivationFunctionType.Copy)
                        u8_i += 1
                    else:
                        kn = min(KCHUNK, KLAST - k0)
                        at_t = atp.tile([128, max_slab], bf16, tag="at")
                        lo = int(PASS_OFF_B16[pi]) + (k0 - KU8) * span
                        dma_engines[slab_i % 2].dma_start(
                            out=at_t[:, :kn * span],
                            in_=ATB_d[:, lo:lo + kn * span])
                    slab_i += 1
                    for kk in range(kn):
                        k = k0 + kk
                        for bi in range(nbank):
                            ncols = min(BANK_COLS, span - bi * BANK_COLS)
                            off = kk * span + bi * BANK_COLS
                            nc.tensor.matmul(
                                ps[bi][:, :ncols],
                                y_tiles[k][:],
                                at_t[:, off:off + ncols],
                                start=(k == 0), stop=False,
                                skip_group_check=True,
                            )
                # short k=78 slab: only the 16 real src rows, bf16
                at2_t = at2p.tile([KLAST_ROWS, GROUPS_PER_PASS * 128], bf16,
                                  tag="at2")
                lo2 = int(PASS_OFF2[pi])
                nc.scalar.dma_start(out=at2_t[:, :span],
                                    in_=AT2_d[:, lo2:lo2 + span])
                for bi in range(nbank):
                    ncols = min(BANK_COLS, span - bi * BANK_COLS)
                    nc.tensor.matmul(
                        ps[bi][:, :ncols],
                        y_tiles[KLAST][:KLAST_ROWS, :],
                        at2_t[:, bi * BANK_COLS:bi * BANK_COLS + ncols],
                        start=False, stop=True,
                        skip_group_check=True,
                    )
                # evict: relu(psum + bias) in one ACT op to bf16, DMA out
                for bi in range(nbank):
                    c0 = col0 + bi * BANK_COLS
                    real = min(BANK_COLS, col0 + span - c0)
                    ot = evp.tile([128, BANK_COLS], bf16, tag="ot")
                    nc.scalar.activation(
                        out=ot[:, :real], in_=ps[bi][:, :real],
                        func=mybir.ActivationFunctionType.Relu,
                        bias=bias_t[:, 0:1])
                    nc.sync.dma_start(out=outT_d[:, c0:c0 + real],
                                      in_=ot[:, :real])

    nc.finalize()
    return nc


# ---------------------------------------------------------------- entry

def kernel(x, edge_rows, edge_cols, edge_vals, W, b):
    x = np.asarray(x)
    in_maps = prep_in_maps(x, edge_rows, edge_cols, edge_vals, W, b)
    nc = build_nc()
    res = run_bass_kernel_spmd(nc, in_maps, list(range(x.shape[0])))
    out = np.stack([np.asarray(r["outT"]).astype(np.float32).T
                    for r in res.results])
    return out


# revision 18
# speedup vs baseline: 1.1113x; 1.1113x over previous
"""GCNN (batched SpMM + GEMM + bias + ReLU) Trainium2 kernel.

Strategy: dense block-streamed SpMM with a mixed uint8/bf16 A^T stream
(no gather, no per-edge DMA descriptors).

Per-core work (one graph per NeuronCore, 8 graphs / 8 cores):
  phase 0: y = x @ W'           (bf16 PE matmuls, y tiles stay in SBUF;
           W' = W * s/255 folds the A-dequant scale so the uint8->bf16
           A^T conversion on-chip is a pure cast)
  phase 1: out^T = A^T-blocks streamed dense:
      out^T[ch, dest] = sum_k y_k^T @ A^T[k-block, dest-block]
    - A^T built dense on host in a pass-major layout [128 src-lane,
      pass | k | dest-col]; k-blocks < KU8 are stored uint8
      (round(v*255/s), s = per-graph max) and dequant-cast to bf16
      on-chip (ACT + DVE, ~2:1), the rest are stored bf16 and used
      directly.  The split ratio balances DMA bytes (~133 MB) against
      the measured cast rates (ACT ~10us, DVE ~22us per 1.57M-elem
      slab) and the PE stream (~390us) so all engines run ~350-400us.
    - the last src block (16 real rows of 128) is streamed as a
      [16, span] bf16 slab - its 112 zero partitions are never moved
    - y_k (bf16, SBUF-resident) is the matmul *stationary* operand;
      bf16 A^T slabs are the *moving* operand
    - PSUM accumulates out^T per dest pass over all 79 k-blocks;
      7 dest passes of 12 groups (3 PSUM banks each) ping-pong across
      6 PSUM banks so evictions overlap the next pass's matmuls
    - eviction: single ACT op relu(psum + bias) (bias is per-partition
      in the out^T orientation) to bf16, DMA out; host transposes and
      upcasts

Quantization error: uint8 absolute quantization of v in [0,1) gives
~0.4% output rel err (vs 2e-2 gate); fp8 e4m3 *relative* quantization
was measured at 2.4e-2 and rejected.  Integers 0..255 are exact in
bf16, so the cast adds no further error.

SPMD: one NEFF for all 8 cores; per-core tensors differ only in data.
"""

import sys

if "/opt/trn_rl_repo" not in sys.path:
    sys.path.insert(0, "/opt/trn_rl_repo")

import numpy as np
import ml_dtypes

import concourse.bacc as bacc
import concourse.mybir as mybir
from concourse import tile
from concourse.bass_utils import run_bass_kernel_spmd

BF16 = ml_dtypes.bfloat16

C = 128            # channels (C_IN == C_OUT == 128)
N = 10000          # nodes per graph
NB = (N + 127) // 128          # 79 src blocks
KLAST = NB - 1                 # last src block: only 16 real rows
KLAST_ROWS = N - KLAST * 128   # 16
KU8 = 72                       # k-blocks [0, KU8) stored uint8, rest bf16
GROUPS_PER_PASS = 12           # 3 PSUM banks per pass, 6 banks ping-pong
BANK_COLS = 512                # fp32 columns per PSUM bank
KCHUNK = 8                     # k-blocks per DMA slab


def _passes():
    out = []
    g = 0
    while g * 128 < N:
        c0 = g * 128
        c1 = min((g + GROUPS_PER_PASS) * 128, N)
        out.append((c0, c1 - c0))          # (first dest col, real span)
        g += GROUPS_PER_PASS
    return out


PASSES = _passes()                          # [(col0, span), ...]
PASS_OFF_U8 = np.concatenate(
    [[0], np.cumsum([KU8 * s for _, s in PASSES])]).astype(np.int64)
ATQ_COLS = int(PASS_OFF_U8[-1])             # 56 * 10000
PASS_OFF_B16 = np.concatenate(
    [[0], np.cumsum([(KLAST - KU8) * s for _, s in PASSES])]).astype(np.int64)
ATB_COLS = int(PASS_OFF_B16[-1])            # 22 * 10000
# the short k=78 slab lives in a separate [16, N] tensor, pass-major too
PASS_OFF2 = np.concatenate(
    [[0], np.cumsum([s for _, s in PASSES])]).astype(np.int64)


# ---------------------------------------------------------------- host prep

def prep_in_maps(x, edge_rows, edge_cols, edge_vals, W, b):
    """Build per-core input maps: xT (bf16), W' (bf16, scale-folded),
    bT (f32 bias in out^T orientation), ATQ (uint8) / ATB (bf16) /
    AT2 (bf16) dense A^T pieces, pass-major."""
    x = np.asarray(x)
    W32 = np.asarray(W, dtype=np.float32)
    bT = np.ascontiguousarray(
        np.asarray(b, dtype=np.float32)[:, None] * np.ones((1, 1), np.float32))

    spans = np.array([s for _, s in PASSES], dtype=np.int64)

    in_maps = []
    for g in range(x.shape[0]):
        rows = np.asarray(edge_rows[g], dtype=np.int64)
        cols = np.asarray(edge_cols[g], dtype=np.int64)
        vals = np.asarray(edge_vals[g], dtype=np.float32)
        k = cols // 128
        c_loc = cols % 128
        p = rows // (GROUPS_PER_PASS * 128)
        r_rel = rows - p * GROUPS_PER_PASS * 128

        mq = k < KU8
        atq = np.zeros(C * ATQ_COLS, dtype=np.float32)
        idx = (c_loc[mq] * ATQ_COLS
               + PASS_OFF_U8[p[mq]] + k[mq] * spans[p[mq]] + r_rel[mq])
        np.add.at(atq, idx, vals[mq])

        mb = (k >= KU8) & (k < KLAST)
        atb = np.zeros(C * ATB_COLS, dtype=np.float32)
        idx = (c_loc[mb] * ATB_COLS
               + PASS_OFF_B16[p[mb]] + (k[mb] - KU8) * spans[p[mb]]
               + r_rel[mb])
        np.add.at(atb, idx, vals[mb])

        m2 = k == KLAST
        at2 = np.zeros(KLAST_ROWS * N, dtype=np.float32)
        idx2 = c_loc[m2] * N + PASS_OFF2[p[m2]] + r_rel[m2]
        np.add.at(at2, idx2, vals[m2])

        s = float(max(atq.max(), 1e-9))
        q = np.clip(np.rint(atq * (255.0 / s)), 0, 255).astype(np.uint8)
        sc = np.float32(s / 255.0)

        in_maps.append({
            "xT": np.ascontiguousarray(x[g].T.astype(BF16)),
            "W": (W32 * sc).astype(BF16),
            "bT": bT,
            "ATQ": q.reshape(C, ATQ_COLS),
            # bf16 pieces carry real values; pre-divide by the folded
            # scale so one W' serves both streams
            "ATB": (atb.reshape(C, ATB_COLS) / sc).astype(BF16),
            "AT2": (at2.reshape(KLAST_ROWS, N) / sc).astype(BF16),
        })
    return in_maps


# ---------------------------------------------------------------- device IR

def build_nc():
    f32 = mybir.dt.float32
    bf16 = mybir.dt.bfloat16
    u8 = mybir.dt.uint8

    nc = bacc.Bacc("TRN2")
    xT_d = nc.dram_tensor("xT", [C, N], bf16, kind="ExternalInput")
    W_d = nc.dram_tensor("W", [C, C], bf16, kind="ExternalInput")
    bT_d = nc.dram_tensor("bT", [C, 1], f32, kind="ExternalInput")
    ATQ_d = nc.dram_tensor("ATQ", [C, ATQ_COLS], u8, kind="ExternalInput")
    ATB_d = nc.dram_tensor("ATB", [C, ATB_COLS], bf16, kind="ExternalInput")
    AT2_d = nc.dram_tensor("AT2", [KLAST_ROWS, N], bf16, kind="ExternalInput")
    outT_d = nc.dram_tensor("outT", [C, N], bf16, kind="ExternalOutput")

    max_slab = KCHUNK * GROUPS_PER_PASS * 128   # elems per partition

    with tile.TileContext(nc) as tc:
        with (
            tc.tile_pool(name="const", bufs=1) as constp,
            tc.tile_pool(name="ypool", bufs=NB) as ypool,
            tc.tile_pool(name="p0ps", bufs=2, space="PSUM") as p0ps,
            tc.tile_pool(name="atq", bufs=3) as atqp,
            tc.tile_pool(name="atp", bufs=3) as atp,
            tc.tile_pool(name="at2p", bufs=2) as at2p,
            tc.tile_pool(name="acc", bufs=6, space="PSUM") as accp,
            tc.tile_pool(name="ev", bufs=3) as evp,
        ):
            # ---- constants
            w_t = constp.tile([C, C], bf16, tag="w")
            nc.sync.dma_start(out=w_t[:], in_=W_d[:])
            bias_t = constp.tile([C, 1], f32, tag="bias")
            nc.sync.dma_start(out=bias_t[:], in_=bT_d[:])
            x_t = constp.tile([C, N], bf16, tag="x")
            nc.scalar.dma_start(out=x_t[:], in_=xT_d[:])

            # ---- phase 0: y = x @ W', tiles kept resident in SBUF (bf16)
            y_tiles = []
            for t in range(NB):
                rows = min(128, N - t * 128)
                yps = p0ps.tile([128, C], f32, tag="yps")
                nc.tensor.matmul(yps[:rows, :],
                                 x_t[:, t * 128:t * 128 + rows],
                                 w_t[:], start=True, stop=True)
                ysb = ypool.tile([128, C], bf16, tag="y", name=f"y_{t}")
                nc.vector.tensor_copy(ysb[:rows, :], yps[:rows, :])
                y_tiles.append(ysb)

            # ---- phase 1: stream A^T slabs (u8 -> cast, or bf16 direct),
            #      accumulate out^T in PSUM
            dma_engines = [nc.sync, nc.scalar]
            slab_i = 0
            u8_i = 0
            for pi, (col0, span) in enumerate(PASSES):
                nbank = (span + BANK_COLS - 1) // BANK_COLS
                ps = []
                for bi in range(nbank):
                    pt = accp.tile([128, BANK_COLS], f32, tag="acc",
                                   name=f"acc_{col0}_{bi}")
                    ps.append(pt)
                for k0 in range(0, KLAST, KCHUNK):
                    if k0 < KU8:
                        kn = min(KCHUNK, KU8 - k0)
                        atq_t = atqp.tile([128, max_slab], u8, tag="atq")
                        lo = int(PASS_OFF_U8[pi]) + k0 * span
                        dma_engines[slab_i % 2].dma_start(
                            out=atq_t[:, :kn * span],
                            in_=ATQ_d[:, lo:lo + kn * span])
                        at_t = atp.tile([128, max_slab], bf16, tag="at")
                        # dequant-cast: alternate ACT / DVE (similar rates
                        # when GPSIMD stays off the shared SBUF port)
                        if u8_i % 2 == 1:
                            nc.vector.tensor_copy(at_t[:, :kn * span],
                                                  atq_t[:, :kn * span])
                        else:
                            nc.scalar.activation(
                                out=at_t[:, :kn * span],
                                in_=atq_t[:, :kn * span],
                                func=mybir.ActivationFunctionType.Copy)
                        u8_i += 1
                    else:
                        kn = min(KCHUNK, KLAST - k0)
                        at_t = atp.tile([128, max_slab], bf16, tag="at")
                        lo = int(PASS_OFF_B16[pi]) + (k0 - KU8) * span
                        dma_engines[slab_i % 2].dma_start(
                            out=at_t[:, :kn * span],
                            in_=ATB_d[:, lo:lo + kn * span])
                    slab_i += 1
                    for kk in range(kn):
                        k = k0 + kk
                        for bi in range(nbank):
                            ncols = min(BANK_COLS, span - bi * BANK_COLS)
                            off = kk * span + bi * BANK_COLS
                            nc.tensor.matmul(
                                ps[bi][:, :ncols],
                                y_tiles[k][:],
                                at_t[:, off:off + ncols],
                                start=(k == 0), stop=False,
                                skip_group_check=True,
                            )
                # short k=78 slab: only the 16 real src rows, bf16
                at2_t = at2p.tile([KLAST_ROWS, GROUPS_PER_PASS * 128], bf16,
                                  tag="at2")
                lo2 = int(PASS_OFF2[pi])
                nc.scalar.dma_start(out=at2_t[:, :span],
                                    in_=AT2_d[:, lo2:lo2 + span])
                for bi in range(nbank):
                    ncols = min(BANK_COLS, span - bi * BANK_COLS)
                    nc.tensor.matmul(
                        ps[bi][:, :ncols],
                        y_tiles[KLAST][:KLAST_ROWS, :],
                        at2_t[:, bi * BANK_COLS:bi * BANK_COLS + ncols],
                        start=False, stop=True,
                        skip_group_check=True,
                    )
                # evict: relu(psum + bias) in one ACT op to bf16, DMA out
                for bi in range(nbank):
                    c0 = col0 + bi * BANK_COLS
                    real = min(BANK_COLS, col0 + span - c0)
                    ot = evp.tile([128, BANK_COLS], bf16, tag="ot")
                    nc.scalar.activation(
                        out=ot[:, :real], in_=ps[bi][:, :real],
                        func=mybir.ActivationFunctionType.Relu,
                        bias=bias_t[:, 0:1])
                    nc.sync.dma_start(out=outT_d[:, c0:c0 + real],
                                      in_=ot[:, :real])

    nc.finalize()
    return nc


# ---------------------------------------------------------------- entry

def kernel(x, edge_rows, edge_cols, edge_vals, W, b):
    x = np.asarray(x)
    in_maps = prep_in_maps(x, edge_rows, edge_cols, edge_vals, W, b)
    nc = build_nc()
    res = run_bass_kernel_spmd(nc, in_maps, list(range(x.shape[0])))
    out = np.stack([np.asarray(r["outT"]).astype(np.float32).T
                    for r in res.results])
    return out
